# revision 38
# baseline (speedup 1.0000x reference)
"""Trainium2 Bass kernel for CausalSelfAttention (GQA, RoPE, prefill).

Tensor-parallel over the 8 query groups: core g owns query heads
[4g, 4g+4) and kv head g.  Each core computes a partial output
(full-shape) that the host sums.

Per-core pipeline (all on one NeuronCore, Tile-scheduled):
  1. qkvT = wqkvT.T @ xT          (fp16 matmuls, feature-major out)
  2. RoPE on q and k (per 512-token half, overlapped with the next
     qkv chunk), v -> token-major via PE transposes
  3. per (batch, head): scores computed KV-MAJOR (scoresT = kT.T @ qT)
     with the causal mask ADDED on the PE (identity-matmul accumulate
     of a -30000 bias into the diagonal block), exp on ACT (scale =
     1/sqrt(hs)) straight into the PV rhs layout.  Row sums via a
     ones-vector matmul on PE; 1/s = exp(-ln(s)) on two tiny [1,512]
     ACT ops, broadcast via a ones-column matmul on PE, one DVE
     multiply normalizes the (already SBUF-copied) unnormalized y.
     Heads are software-pipelined two deep: scores+exp of head i+1 are
     emitted before the PV of head i so ACT and PE overlap; the
     normalize of head i-1 fills the remaining gaps.
  4. out_partial = yT.T @ wprojT  (fp16 matmuls), f16 partial output.
"""

import os

import numpy as np

B, T, NE, NH, NQG, HS = 2, 1024, 4096, 32, 8, 128
QPK = NH // NQG          # 4 query heads per kv group
NT = B * T               # 2048 tokens
GW = (QPK + 2) * HS      # 768 qkv rows per group
GQ = QPK * HS            # 512 q cols per group
P = 128
NCORES = 8
KC = NE // P             # 32 contraction chunks for qkv proj
MC = GW // P             # 6 qkv feature chunks
TC8 = T // P             # 8 token chunks per batch
MASKNEG = -30000.0       # additive causal bias (fits f16; exp -> exact 0)
SCALE = 1.0 / float(np.sqrt(HS))

_CACHE = {}


def _split_waits(nc, mybir, max_waits=1):
    """walrus in this container rejects >1 sync-wait per instruction;
    hoist extras onto single-wait NoOps just before (equivalent since
    semaphores are monotonic and a sequencer executes in order)."""
    for fn in nc.m.functions:
        for blk in fn.blocks:
            new_list, changed = [], False
            for inst in blk.instructions:
                si = getattr(inst, "sync_info", None)
                if si is not None and len(si.on_wait) > max_waits:
                    waits = list(si.on_wait)
                    for i, w in enumerate(waits[:-max_waits]):
                        nop = mybir.InstNoOp(
                            name=f"{inst.name}-wsplit-{i}", ins=[], outs=[],
                            engine=inst.engine)
                        nop.sync_info = mybir.SyncInfo(on_wait=[w], on_update=[])
                        new_list.append(nop)
                    inst.sync_info = mybir.SyncInfo(
                        on_wait=waits[-max_waits:], on_update=list(si.on_update))
                    changed = True
                new_list.append(inst)
            if changed:
                blk.instructions = new_list


def _build_nc(reps=1, debug=False):
    import concourse.bass as bass
    import concourse.mybir as mybir
    import concourse.tile as tile
    from contextlib import ExitStack

    f32 = mybir.dt.float32
    f16 = mybir.dt.float16
    Exp = mybir.ActivationFunctionType.Exp
    Ln = mybir.ActivationFunctionType.Ln

    nc = bass.Bass()
    xT_d = nc.dram_tensor("xT", [NE, NT], f16, kind="ExternalInput")
    wqkvT_d = nc.dram_tensor("wqkvT", [NE, GW], f16, kind="ExternalInput")
    wprojT_d = nc.dram_tensor("wprojT", [GQ, NE], f16, kind="ExternalInput")
    cc_d = nc.dram_tensor("cc", [P, T], f32, kind="ExternalInput")
    ss_d = nc.dram_tensor("ss", [P, T], f32, kind="ExternalInput")
    maskb_d = nc.dram_tensor("maskb", [P, P], f16, kind="ExternalInput")
    ones16_d = nc.dram_tensor("ones16", [P, 1], f16, kind="ExternalInput")
    onesbc_d = nc.dram_tensor("onesbc", [1, P], f16, kind="ExternalInput")
    ident16_d = nc.dram_tensor("ident16", [P, P], f16, kind="ExternalInput")
    out_d = nc.dram_tensor("out", [NT, NE], f16, kind="ExternalOutput")
    if debug:
        tap_q_d = nc.dram_tensor("tap_q", [P, QPK, NT], f16, kind="ExternalOutput")
        tap_k_d = nc.dram_tensor("tap_k", [P, NT], f16, kind="ExternalOutput")
        tap_v_d = nc.dram_tensor("tap_v", [P, B * TC8, P], f16, kind="ExternalOutput")
        tap_e_d = nc.dram_tensor("tap_e", [P, 4608], f16, kind="ExternalOutput")
        tap_l_d = nc.dram_tensor("tap_l", [2, 512], f16, kind="ExternalOutput")
        tap_r_d = nc.dram_tensor("tap_r", [P, 512], f16, kind="ExternalOutput")
        tap_y_d = nc.dram_tensor("tap_y", [P, QPK, NT], f16, kind="ExternalOutput")

    # column offset of kv-chunk c's block inside the expT tile
    offs, acc = [], 0
    for c in range(TC8):
        offs.append(acc)
        acc += (TC8 - c) * P

    with tile.TileContext(nc) as tc:
      for _rep in range(reps):
        sL = ExitStack()   # left-side long-lived pools (y, wp, ob)
        sR = ExitStack()   # right-side pools (qk16, attention-era)
        try:
            # const: left
            const = sL.enter_context(tc.tile_pool(name="const", bufs=1))
            cc = const.tile([P, T], f32)
            ss = const.tile([P, T], f32)
            maskb = const.tile([P, P], f16)
            ones16 = const.tile([P, 1], f16)
            onesbc = const.tile([1, P], f16)
            ident16 = const.tile([P, P], f16)

            # qk16 on the right: lives through attention
            qk16 = sR.enter_context(tc.tile_pool(name="qk16", bufs=1, side="right"))
            q16 = qk16.tile([P, QPK, NT], f16)
            k16 = qk16.tile([P, NT], f16)
            vtm = qk16.tile([P, B * TC8, P], f16)

            # ============ phase 1+2: qkv projection + rope ========
            with ExitStack() as sA:
                qkv_pool = sA.enter_context(tc.tile_pool(name="qkv", bufs=1))
                qkv = qkv_pool.tile([P, MC, NT], f16)
                wq_pool = sA.enter_context(tc.tile_pool(name="wq", bufs=1))
                wq = wq_pool.tile([P, KC, GW], f16)
                wqr = wqkvT_d[:].rearrange("(ko p) m -> p ko m", p=P)
                xs_pool = sA.enter_context(tc.tile_pool(name="xs", bufs=16))
                ps1 = sA.enter_context(
                    tc.tile_pool(name="ps1", bufs=6, space="PSUM"))
                rp = sA.enter_context(tc.tile_pool(name="rope", bufs=2))

                def emit_rope(b, h):
                    # RoPE for batch b, 512-token half h (rotate-half form)
                    hh = HS // 2
                    gl = slice(b * T + h * 512, b * T + (h + 1) * 512)
                    ccb = cc[:, h * 512:(h + 1) * 512]
                    ssb = ss[:, h * 512:(h + 1) * 512]
                    for hc in range(QPK + 1):
                        src = qkv[:, hc, gl]
                        rot = rp.tile([P, 512], f16, tag="rot",
                                      name=f"rot{b}_{h}_{hc}")
                        nc.gpsimd.dma_start(rot[0:hh, :], src[hh:P, :])
                        nc.gpsimd.dma_start(rot[hh:P, :], src[0:hh, :])
                        t1 = rp.tile([P, 512], f32, tag="t1",
                                     name=f"t1_{b}_{h}_{hc}")
                        t2 = rp.tile([P, 512], f32, tag="t2",
                                     name=f"t2_{b}_{h}_{hc}")
                        nc.vector.tensor_mul(t1[:], src, ccb)
                        nc.vector.tensor_mul(t2[:], rot[:], ssb)
                        dst = q16[:, hc, gl] if hc < QPK else k16[:, gl]
                        nc.vector.tensor_add(dst, t1[:], t2[:])
                    for c in range(4 * h, 4 * h + 4):
                        # PE transpose of v into token-major
                        vt_ps = ps1.tile([P, P], f16, tag="vt", bufs=2,
                                         name=f"vt{b}_{c}")
                        nc.tensor.transpose(
                            vt_ps[:],
                            qkv[:, QPK + 1, b * T + c * P: b * T + (c + 1) * P],
                            ident16[:])
                        nc.any.tensor_copy(vtm[:, b * TC8 + c, :], vt_ps[:])

                # consts first (gpsimd queue), then PE warmup matmuls on
                # ident16: keeps the PE busy during the initial wq/x DMA
                # latency and brings HAM to 8/8 before the real QKV work.
                nc.sync.dma_start(ident16[:], ident16_d[:])
                nc.gpsimd.dma_start(cc[:], cc_d[:])
                nc.gpsimd.dma_start(ss[:], ss_d[:])
                nc.gpsimd.dma_start(maskb[:], maskb_d[:])
                nc.gpsimd.dma_start(ones16[:], ones16_d[:])
                nc.gpsimd.dma_start(onesbc[:], onesbc_d[:])
                for w_ in range(6):
                    warm = ps1.tile([P, 512], f32, tag="ps1",
                                    name=f"warm{w_}")
                    for v_ in range(8):
                        nc.tensor.matmul(warm[:, 0:P], ident16[:],
                                         ident16[:], start=True, stop=True)

                for b in range(B):
                    for n in (2 * b, 2 * b + 1):
                        psums = [ps1.tile([P, 512], f32, tag="ps1",
                                          name=f"ps1_{n}_{m_}")
                                 for m_ in range(MC)]
                        for k in range(KC):
                            if n == 0:
                                nc.scalar.dma_start(wq[:, k, :], wqr[:, k, :])
                            xt = xs_pool.tile([P, 512], f16, tag="xt",
                                              name=f"xt{n}_{k}")
                            nc.sync.dma_start(
                                xt[:], xT_d[k * P:(k + 1) * P,
                                            n * 512:(n + 1) * 512])
                            for m in range(MC):
                                nc.tensor.matmul(
                                    psums[m][:], wq[:, k, m * P:(m + 1) * P],
                                    xt[:], start=(k == 0), stop=(k == KC - 1))
                        for m in range(MC):
                            nc.any.tensor_copy(
                                qkv[:, m, n * 512:(n + 1) * 512], psums[m][:])
                        emit_rope(b, n - 2 * b)

            # ============ phases 3+4 pools ============
            y_pool = sL.enter_context(tc.tile_pool(name="y", bufs=1))
            y_sb = y_pool.tile([P, QPK, NT], f16)
            wp_pool = sL.enter_context(tc.tile_pool(name="wp", bufs=1))
            wp = wp_pool.tile([P, QPK, NE], f16)
            wpr = wprojT_d[:].rearrange("(kc p) n -> p kc n", p=P)
            for kc in range(QPK):
                nc.sync.dma_start(wp[:, kc, :], wpr[:, kc, :])
            ob_pool = sL.enter_context(tc.tile_pool(name="ob", bufs=2))

            expT_pool = sR.enter_context(
                tc.tile_pool(name="expT", bufs=2, side="right"))
            stat_pool = sR.enter_context(
                tc.tile_pool(name="stat", bufs=4, side="right"))
            rb_pool = sR.enter_context(
                tc.tile_pool(name="rb", bufs=4, side="right"))
            psA = sR.enter_context(tc.tile_pool(name="psA", bufs=1, space="PSUM"))

            # ============ phase 3: attention ============
            # Two-stage head pipeline: scores+exp of head i+1 are emitted
            # BEFORE the PV of head i, so the ACT exps of the next head
            # overlap the PE's PV matmuls of the current one.
            heads = [(b, hc) for b in range(B) for hc in range(QPK)]
            expTs = {}          # idx -> expT tile
            pending = {}        # idx -> list of (yps, rbrow, dest, key)

            def emit_scores(idx):
                b, hc = heads[idx]
                qT_i = q16[:, hc, b * T:(b + 1) * T]
                expT = expT_pool.tile([P, acc], f16, tag="expT",
                                      name=f"expT{b}_{hc}")
                expTs[idx] = expT
                for c in range(TC8):
                    kT_c = k16[:, b * T + c * P: b * T + (c + 1) * P]
                    e1 = 512 if c < 4 else T
                    w1 = e1 - c * P
                    sps = psA.tile([P, 512], f32, tag="acc", bufs=4,
                                   name=f"sps{b}_{hc}_{c}")
                    # full-width scores first (start clears the whole
                    # bank); both kT_c matmuls back-to-back so the ident
                    # LDWEIGHTS for the mask hides under the second
                    # stream, then accumulate the causal bias onto the
                    # diagonal 128 columns
                    nc.tensor.matmul(sps[:, 0:w1], kT_c,
                                     qT_i[:, c * P:e1],
                                     start=True, stop=False)
                    sps2 = None
                    if c < 4:
                        sps2 = psA.tile([P, 512], f32, tag="acc", bufs=4,
                                        name=f"sps2_{b}_{hc}_{c}")
                        nc.tensor.matmul(sps2[:], kT_c, qT_i[:, 512:T],
                                         start=True, stop=True)
                    nc.tensor.matmul(sps[:, 0:P], ident16[:], maskb[:],
                                     start=False, stop=True)
                    nc.scalar.activation(
                        expT[:, offs[c]:offs[c] + w1], sps[:, :w1],
                        Exp, scale=SCALE)
                    if c < 4:
                        nc.scalar.activation(
                            expT[:, offs[c] + w1:offs[c] + w1 + 512],
                            sps2[:], Exp, scale=SCALE)
                if debug and idx == 0:
                    nc.sync.dma_start(tap_e_d[:], expT[:])

            def emit_pv(idx):
                b, hc = heads[idx]
                expT = expTs.pop(idx)
                pend = pending.setdefault(idx, [])
                for (s0, s1) in ((0, 512), (512, T)):
                    yps = psA.tile([P, 512], f32, tag="yps", bufs=2,
                                   name=f"yps{b}_{hc}_{s0}")
                    s_ps = psA.tile([1, 512], f32, tag="s", bufs=2,
                                    name=f"s_{b}_{hc}_{s0}")
                    cs = [c for c in range(TC8) if c * P < s1]
                    for c in cs:
                        q0 = max(s0, c * P)
                        sl = slice(offs[c] + (q0 - c * P),
                                   offs[c] + (s1 - c * P))
                        nc.tensor.matmul(
                            yps[:, q0 - s0:s1 - s0],
                            vtm[:, b * TC8 + c, :], expT[:, sl],
                            start=(c == cs[0]), stop=(c == cs[-1]))
                        nc.tensor.matmul(
                            s_ps[:, q0 - s0:s1 - s0], ones16[:],
                            expT[:, sl],
                            start=(c == cs[0]), stop=(c == cs[-1]))
                    # unnormalized y -> SBUF f16 (frees the PSUM bank early)
                    y_un = rb_pool.tile([P, 512], f16, tag="yun",
                                        name=f"yun{b}_{hc}_{s0}")
                    nc.vector.tensor_copy(y_un[:], yps[:])
                    # 1/s = exp(-ln(s)), both on tiny [1,512] ACT ops
                    lsb = stat_pool.tile([1, 512], f16, tag="lsb",
                                         name=f"lsb{b}_{hc}_{s0}")
                    nc.scalar.activation(lsb[:], s_ps[:], Ln)
                    rbrow = stat_pool.tile([1, 512], f16, tag="rbrow",
                                           name=f"rbr{b}_{hc}_{s0}")
                    nc.scalar.activation(rbrow[:], lsb[:], Exp, scale=-1.0)
                    pend.append(
                        (y_un, rbrow,
                         y_sb[:, hc, b * T + s0:b * T + s1],
                         f"{b}_{hc}_{s0}"))

            def emit_finish(idx):
                # broadcast 1/s on the PE, then one DVE multiply
                for y_un, rbrow, dest, key in pending.pop(idx):
                    bc_ps = psA.tile([P, 512], f32, tag="acc", bufs=4,
                                     name=f"bc{key}")
                    nc.tensor.matmul(bc_ps[:], onesbc[:], rbrow[:],
                                     start=True, stop=True)
                    nc.vector.tensor_mul(dest, y_un[:], bc_ps[:])
                    if debug and key == "0_0_0":
                        nc.sync.dma_start(tap_l_d[0:1, :], rbrow[:])
                        nc.sync.dma_start(tap_r_d[:], y_un[:])

            emit_scores(0)
            for i in range(len(heads)):
                if i + 1 < len(heads):
                    emit_scores(i + 1)
                if i - 1 >= 0:
                    emit_finish(i - 1)
                emit_pv(i)
            emit_finish(len(heads) - 1)
            if debug:
                nc.sync.dma_start(tap_q_d[:], q16[:]);
                nc.sync.dma_start(tap_k_d[:], k16[:])
                nc.sync.dma_start(tap_v_d[:], vtm[:])
                nc.sync.dma_start(tap_y_d[:], y_sb[:])

            # ============ phase 4: output projection ============
            for m in range(NT // P):
                ob = ob_pool.tile([P, NE], f16, tag="ob", name=f"ob{m}")
                for n in range(NE // 512):
                    opsum = psA.tile([P, 512], f32, tag="acc", bufs=4,
                                     name=f"ops{m}_{n}")
                    for kc in range(QPK):
                        nc.tensor.matmul(
                            opsum[:], y_sb[:, kc, m * P:(m + 1) * P],
                            wp[:, kc, n * 512:(n + 1) * 512],
                            start=(kc == 0), stop=(kc == QPK - 1))
                    nc.any.tensor_copy(ob[:, n * 512:(n + 1) * 512], opsum[:])
                    if n % 2 == 1:
                        nc.sync.dma_start(
                            out_d[m * P:(m + 1) * P,
                                  (n - 1) * 512:(n + 1) * 512],
                            ob[:, (n - 1) * 512:(n + 1) * 512])
        finally:
            sR.close()
            sL.close()

    _split_waits(nc, mybir)
    return nc


def _host_prep(x, cos, sin, W_attn, W_proj):
    xT = np.ascontiguousarray(x.reshape(NT, NE).T.astype(np.float16))
    cc = np.ascontiguousarray(
        np.concatenate([cos.T, cos.T], axis=0), dtype=np.float32)
    ss = np.ascontiguousarray(
        np.concatenate([-sin.T, sin.T], axis=0), dtype=np.float32)
    # scoresT layout [kv, q]: additive -30000 on strictly-lower (kv > q)
    maskb = np.where(np.tri(P, P, -1, dtype=bool),
                     np.float16(MASKNEG), np.float16(0.0))
    common = {"xT": xT, "cc": cc, "ss": ss, "maskb": maskb,
              "ones16": np.ones((P, 1), dtype=np.float16),
              "ident16": np.eye(P, dtype=np.float16),
              "onesbc": np.ones((1, P), dtype=np.float16)}
    in_maps = []
    for g in range(NCORES):
        m = dict(common)
        m["wqkvT"] = np.ascontiguousarray(
            W_attn[g * GW:(g + 1) * GW, :].T.astype(np.float16))
        m["wprojT"] = np.ascontiguousarray(
            W_proj[:, g * GQ:(g + 1) * GQ].T.astype(np.float16))
        in_maps.append(m)
    return in_maps


LAST_EXEC_NS = None


def _maybe_trace_kwargs():
    """If BASS_TRACE is set, shim antenv.axon_hooks (absent in this image)
    so run_bass_kernel_spmd can NTFF-profile and report exec_time_ns."""
    if not os.environ.get("BASS_TRACE"):
        return {}
    try:
        import sys
        import types
        if "antenv.axon_hooks" not in sys.modules:
            mod = types.ModuleType("antenv.axon_hooks")
            mod._hook = None
            mod.set_axon_ntff_profile_hook = lambda h: setattr(mod, "_hook", h)
            mod.get_axon_ntff_profile_hook = lambda: mod._hook
            sys.modules["antenv.axon_hooks"] = mod
            from trn_agent_boot.trn_boot import _ntff_profile_via_ctypes
            mod._hook = _ntff_profile_via_ctypes("/opt/axon/libaxon_pjrt.so")
        return {"trace": True,
                "trace_cores": list(range(NCORES))}
    except Exception:
        return {}


def kernel(x, cos, sin, W_attn, W_proj, max_seq_length):
    global LAST_EXEC_NS
    from concourse.bass_utils import run_bass_kernel_spmd

    x = np.asarray(x, dtype=np.float32)
    cos = np.asarray(cos, dtype=np.float32)
    sin = np.asarray(sin, dtype=np.float32)
    W_attn = np.asarray(W_attn, dtype=np.float32)
    W_proj = np.asarray(W_proj, dtype=np.float32)

    if "nc" not in _CACHE:
        _CACHE["nc"] = _build_nc()
    nc = _CACHE["nc"]

    in_maps = _host_prep(x, cos, sin, W_attn, W_proj)
    res = run_bass_kernel_spmd(nc, in_maps, core_ids=list(range(NCORES)),
                               **_maybe_trace_kwargs())
    if res.exec_time_ns is not None:
        LAST_EXEC_NS = res.exec_time_ns

    acc = res.results[0]["out"].astype(np.float32)
    for g in range(1, NCORES):
        acc = acc + res.results[g]["out"].astype(np.float32)
    return acc.reshape(B, T, NE)
